# revision 8
# baseline (speedup 1.0000x reference)
"""Trainium2 Bass kernel for 24-rotation (octahedral) 3D conv (ConvZ3P24).

Problem: x (2,4,64,64,64) f32, weight (8,4,3,3,3), bias (8,)
  -> y (2,24,8,64,64,64):  conv3d(x, rotated_filter_bank) + bias,
  stride 1, pad 1, 24 proper octahedral rotations x 8 cout = 192 channels.

Sharding: 8 cores = batch(2) x depth-chunks(4 x 16 planes). Each core
computes all 192 channels for its 16 output planes.

Device kernel (per core), conv as implicit GEMM with K=108 packed taps,
HBM traffic minimized two ways vs the naive im2col formulation:
  1. mod-3 depth ring: the rhs [108, SLOT] SBUF buffer is 3 partition
     blocks of 36 rows (kh,kw,ci); block b holds the (kh,kw)-shifted
     copies of padded input plane q with q === b (mod 3). Each loaded
     plane-block serves 3 consecutive output planes, so the host-side
     im2col only replicates 9x (kh,kw), not 27x: xs9 [36, 18, 4222]
     (5.5 MB/core vs 14.6 MB). The lhsT has 3 column variants r = d%3
     with the kd blocks permuted to match the ring phase.
  2. bf16 output: y is stored bf16 (25.2 MB/core vs 50.3 MB f32) and
     upcast on the host. Adds <=0.4% scale-relative error; gate is 2e-2.
  - per (plane d, channel-half mh, h-block hb): ONE bf16 matmul
    [108,96]^T @ [108, 8x64] -> PSUM [96,512] (fp32 accumulate).
  - PSUM evacuated with fused per-channel bias add, alternating DVE/ACT,
    into a [96, 4096] bf16 stage tile; one 768 KB DMA per (d, mh) on the
    ACT HWDGE ring writes y[mh*96:(mh+1)*96, d].
"""

import itertools
from contextlib import ExitStack

import numpy as np

# ---------------------------------------------------------------- constants
CIN = 4
COUT = 8
N_ROT = 24
KS = 3
DHW = 64
PH = 66            # padded plane side
PLANE = PH * PH    # 4356
SLOT = 4224        # ring slot elems per partition (= 64*66)
VALID = 4222       # elems actually written per partition (max read idx 4221)
N_CORES = 8
DCHUNK = 16        # output planes per core
SLAB_D = 18        # input padded planes per core (16 + 2 halo)
M = 192            # total output channels (24 rot * 8 cout)
MHALF = 96
NTAP = 36          # (kh,kw,ci) rows per kd  == ring block height
HB = 8             # h-blocks per plane
NCOL = 512         # matmul free dim (8 h-rows * 64)

_CACHE = {}


def _rot_index_maps(k):
    """Source voxel indices (d,h,w) for the 24 proper octahedral rotations."""
    m = (k - 1) // 2
    mats = []
    for perm in itertools.permutations(range(3)):
        for signs in itertools.product([1, -1], repeat=3):
            R = np.zeros((3, 3))
            for i in range(3):
                R[i, perm[i]] = signs[i]
            if np.linalg.det(R) > 0.5:
                mats.append(R)
    c = np.arange(k) - m
    cz, cy, cx = np.meshgrid(c, c, c, indexing="ij")
    v = np.stack([cx, cy, cz], 0).astype(np.float64)
    idx = []
    for R in mats:
        g = np.einsum("ij,jdhw->idhw", R, v)
        idx.append(
            np.stack([g[2] + m, g[1] + m, g[0] + m], 0).round().astype(np.int64)
        )
    return np.stack(idx, 0)  # (24, 3, k, k, k)


def _build_program(repeat=1, mode="full"):
    import concourse.bacc as bacc
    import concourse.bass as bass
    import concourse.mybir as mybir
    import concourse.tile as tile

    f32 = mybir.dt.float32
    nc = bacc.Bacc(
        "TRN2",
        target_bir_lowering=False,
        debug=False,
        enable_asserts=False,
        num_devices=N_CORES,
    )

    bf16 = mybir.dt.bfloat16
    xs = nc.dram_tensor(
        "xs", (NTAP, SLAB_D, VALID), bf16, kind="ExternalInput"
    )
    w3 = nc.dram_tensor("w3", (KS * NTAP, 3 * M), bf16, kind="ExternalInput")
    bias2 = nc.dram_tensor("bias2", (MHALF, 2), f32, kind="ExternalInput")
    y = nc.dram_tensor("y", (M, DCHUNK, DHW, DHW), bf16, kind="ExternalOutput")
    xs_ap = xs.ap()
    y_ap = y.ap()

    with tile.TileContext(nc) as tc:
        with ExitStack() as ctx:
            wpool = ctx.enter_context(tc.tile_pool(name="wpool", bufs=1))
            psum = ctx.enter_context(
                tc.tile_pool(name="psum", bufs=8, space="PSUM")
            )
            stage = ctx.enter_context(tc.tile_pool(name="stage", bufs=6))

            # ---- weights + bias (tiny, once)
            w3_t = wpool.tile([KS * NTAP, 3 * M], bf16)
            nc.sync.dma_start(w3_t[:], w3.ap())
            bias_t = wpool.tile([MHALF, 2], f32)
            nc.sync.dma_start(bias_t[:], bias2.ap())
            # two persistent mod-3 rings (3 partition blocks of NTAP rows
            # each): A covers output planes 0-7 (inputs 0-9), B covers
            # 8-15 (inputs 8-17). Interleaving A/B planes hides each
            # ring's reload latency behind the other ring's matmuls.
            rbufA = wpool.tile([KS * NTAP, SLOT], bf16, name="rbufA")
            rbufB = wpool.tile([KS * NTAP, SLOT], bf16, name="rbufB")

            loop_ctx = ExitStack()
            if repeat > 1:
                loop_ctx.enter_context(
                    tc.For_i(
                        0,
                        repeat,
                        1,
                        hint_engines=(
                            mybir.EngineType.PE,
                            mybir.EngineType.DVE,
                            mybir.EngineType.Activation,
                            mybir.EngineType.SP,
                            mybir.EngineType.Pool,
                        ),
                    )
                )

            def load_block(buf, q):
                b = q % 3
                nc.sync.dma_start(
                    buf[NTAP * b : NTAP * (b + 1), 0:VALID], xs_ap[:, q, :]
                )

            noload = mode.endswith("noload")

            for q in range(3):
                load_block(rbufA, q)
            for q in range(8, 11):
                load_block(rbufB, q)

            rhsA = rbufA[:].rearrange("p (h w) -> p h w", w=PH)
            rhsB = rbufB[:].rearrange("p (h w) -> p h w", w=PH)

            def emit_plane(rhs1, d):
                r = d % 3
                for mh in range(2):
                    st = stage.tile([MHALF, HB * NCOL], bf16, tag="stage")
                    for hb in range(HB):
                        ps = psum.tile([MHALF, NCOL], f32, tag="ps")
                        lhsT = w3_t[
                            :, r * M + mh * MHALF : r * M + (mh + 1) * MHALF
                        ]
                        rhs = rhs1[:, hb * HB : hb * HB + HB, 0:DHW]
                        nc.tensor.matmul(
                            ps[:],
                            lhsT,
                            rhs,
                            start=True,
                            stop=True,
                        )
                        if mode.startswith("mm"):
                            continue
                        if mode != "noevac":
                            dst = st[:, hb * NCOL : (hb + 1) * NCOL]
                            if (mh * HB + hb) % 2 == 0:
                                nc.vector.tensor_scalar_add(
                                    dst, ps[:], bias_t[:, mh : mh + 1]
                                )
                            else:
                                nc.scalar.activation(
                                    dst,
                                    ps[:],
                                    mybir.ActivationFunctionType.Identity,
                                    bias=bias_t[:, mh : mh + 1],
                                )
                    if mode == "noevac":
                        # keep the stage tile written so the out-DMA has data
                        nc.vector.tensor_scalar_add(
                            st[:, 0:NCOL], ps[:], bias_t[:, mh : mh + 1]
                        )
                    if not mode.startswith("mm") and mode != "noout":
                        nc.scalar.dma_start(
                            y_ap[mh * MHALF : (mh + 1) * MHALF, d], st[:]
                        )

            for j in range(8):
                emit_plane(rhsA, j)
                if j + 3 <= 9 and not noload:
                    load_block(rbufA, j + 3)
                emit_plane(rhsB, 8 + j)
                if j + 3 <= 9 and not noload:
                    load_block(rbufB, 11 + j)

            loop_ctx.close()

    nc.compile()
    return nc


def _make_runner(nc):
    """Build a reusable jitted SPMD executor (no donation so device buffers
    can be reused across timing calls). Modeled on bass2jax.run_bass_via_pjrt."""
    import jax
    import numpy as _np
    from jax.sharding import Mesh, PartitionSpec
    from jax.experimental.shard_map import shard_map

    import concourse.mybir as mybir
    from concourse import bass2jax

    bass2jax.install_neuronx_cc_hook()

    partition_name = (
        nc.partition_id_tensor.name if nc.partition_id_tensor else None
    )
    in_names, out_names, out_avals, zero_outs = [], [], [], []
    for alloc in nc.m.functions[0].allocations:
        if not isinstance(alloc, mybir.MemoryLocationSet):
            continue
        name = alloc.memorylocations[0].name
        if alloc.kind == "ExternalInput":
            if name != partition_name:
                in_names.append(name)
        elif alloc.kind == "ExternalOutput":
            shape = tuple(alloc.tensor_shape)
            dtype = mybir.dt.np(alloc.dtype)
            out_names.append(name)
            out_avals.append(jax.core.ShapedArray(shape, dtype))
            zero_outs.append(_np.zeros(shape, dtype))
    n_params = len(in_names)
    all_names = in_names + out_names
    if partition_name is not None:
        all_names = all_names + [partition_name]

    def _body(*args):
        operands = list(args)
        if partition_name is not None:
            operands.append(bass2jax.partition_id_tensor())
        outs = bass2jax._bass_exec_p.bind(
            *operands,
            out_avals=tuple(out_avals),
            in_names=tuple(all_names),
            out_names=tuple(out_names),
            lowering_input_output_aliases=(),
            sim_require_finite=True,
            sim_require_nnan=True,
            nc=nc,
        )
        return tuple(outs)

    devices = jax.devices()[:N_CORES]
    mesh = Mesh(np.asarray(devices), ("core",))
    n_args = n_params + len(out_names)
    sharded = jax.jit(
        shard_map(
            _body,
            mesh=mesh,
            in_specs=(PartitionSpec("core"),) * n_args,
            out_specs=(PartitionSpec("core"),) * len(out_names),
            check_rep=False,
        ),
        keep_unused=True,
    )

    from jax.sharding import NamedSharding

    shard = NamedSharding(mesh, PartitionSpec("core"))

    def place_inputs(in_maps):
        """Device-put per-core inputs (sharded along axis 0) + cached zero
        output buffers; returns the full arg list, all device-resident."""
        concat = [
            np.concatenate([np.asarray(m[name]) for m in in_maps], axis=0)
            for name in in_names
        ]
        placed = [jax.device_put(a, shard) for a in concat]
        if "zeros" not in _CACHE:
            _CACHE["zeros"] = [
                jax.device_put(
                    np.zeros((N_CORES * z.shape[0], *z.shape[1:]), z.dtype), shard
                )
                for z in zero_outs
            ]
        return placed + _CACHE["zeros"]

    def run(args):
        return sharded(*args)

    return {
        "place_inputs": place_inputs,
        "run": run,
        "out_names": out_names,
        "out_avals": out_avals,
    }


def _get_runner():
    if "runner" not in _CACHE:
        nc = _build_program()
        _CACHE["runner"] = _make_runner(nc)
    return _CACHE["runner"]


def _get_timing_runner(repeat, mode="full"):
    key = f"runner_r{repeat}_{mode}"
    if key not in _CACHE:
        nc = _build_program(repeat=repeat, mode=mode)
        _CACHE[key] = _make_runner(nc)
    return _CACHE[key]


def _host_prep(x, weight, bias):
    import ml_dtypes

    bf16 = ml_dtypes.bfloat16
    idx = _rot_index_maps(KS)
    wr = weight[:, :, idx[:, 0], idx[:, 1], idx[:, 2]]  # (8,4,24,3,3,3)
    wr = np.transpose(wr, (2, 0, 1, 3, 4, 5)).reshape(M, CIN, KS, KS, KS)
    # lhsT variant r = d%3: ring block b holds kd = (b - r) % 3; rows within
    # a block ordered (kh, kw, ci) to match the xs9 partition order.
    lhsT3 = np.empty((3, KS * NTAP, M), dtype=np.float32)
    for r in range(3):
        for b in range(3):
            kd = (b - r) % 3
            lhsT3[r, NTAP * b : NTAP * (b + 1)] = (
                wr[:, :, kd].transpose(2, 3, 1, 0).reshape(NTAP, M)
            )
    w3 = np.ascontiguousarray(
        lhsT3.transpose(1, 0, 2).reshape(KS * NTAP, 3 * M)
    ).astype(bf16)
    bias192 = np.broadcast_to(bias[None, :], (N_ROT, COUT)).reshape(M)
    bias2 = np.ascontiguousarray(bias192.reshape(2, MHALF).T, dtype=np.float32)

    x_pad = np.zeros((2, CIN, PH, PH, PH), dtype=bf16)
    x_pad[:, :, 1:65, 1:65, 1:65] = x.astype(bf16)

    in_maps = []
    for core in range(N_CORES):
        n, dc = divmod(core, N_CORES // 2)
        flat = x_pad[n, :, DCHUNK * dc : DCHUNK * dc + SLAB_D].reshape(
            CIN, SLAB_D, PLANE
        )
        col9 = np.empty((KS * KS, CIN, SLAB_D, VALID), dtype=bf16)
        for kh in range(KS):
            for kw in range(KS):
                off = kh * PH + kw
                col9[kh * KS + kw] = flat[:, :, off : off + VALID]
        xs9 = np.ascontiguousarray(col9.reshape(NTAP, SLAB_D, VALID))
        in_maps.append({"xs": xs9, "w3": w3, "bias2": bias2})
    return in_maps


def kernel(x, weight, bias):
    x = np.asarray(x, dtype=np.float32)
    weight = np.asarray(weight, dtype=np.float32)
    bias = np.asarray(bias, dtype=np.float32)

    runner = _get_runner()
    in_maps = _host_prep(x, weight, bias)
    args = runner["place_inputs"](in_maps)
    out = runner["run"](args)
    y8 = np.asarray(out[0]).reshape(N_CORES, M, DCHUNK, DHW, DHW)

    yfull = np.empty((2, M, DHW, DHW, DHW), dtype=np.float32)
    for core in range(N_CORES):
        n, dc = divmod(core, N_CORES // 2)
        yfull[n, :, DCHUNK * dc : DCHUNK * (dc + 1)] = y8[core]
    return yfull.reshape(2, N_ROT, COUT, DHW, DHW, DHW)


# revision 9
# speedup vs baseline: 1.0004x; 1.0004x over previous
"""Trainium2 Bass kernel for 24-rotation (octahedral) 3D conv (ConvZ3P24).

Problem: x (2,4,64,64,64) f32, weight (8,4,3,3,3), bias (8,)
  -> y (2,24,8,64,64,64):  conv3d(x, rotated_filter_bank) + bias,
  stride 1, pad 1, 24 proper octahedral rotations x 8 cout = 192 channels.

Sharding: 8 cores = batch(2) x depth-chunks(4 x 16 planes). Each core
computes all 192 channels for its 16 output planes.

Device kernel (per core), conv as implicit GEMM with K=108 packed taps,
HBM traffic minimized two ways vs the naive im2col formulation:
  1. mod-3 depth ring: the rhs [108, SLOT] SBUF buffer is 3 partition
     blocks of 36 rows (kh,kw,ci); block b holds the (kh,kw)-shifted
     copies of padded input plane q with q === b (mod 3). Each loaded
     plane-block serves 3 consecutive output planes, so the host-side
     im2col only replicates 9x (kh,kw), not 27x: xs9 [36, 18, 4222]
     (5.5 MB/core vs 14.6 MB). The lhsT has 3 column variants r = d%3
     with the kd blocks permuted to match the ring phase.
  2. bf16 output: y is stored bf16 (25.2 MB/core vs 50.3 MB f32) and
     upcast on the host. Adds <=0.4% scale-relative error; gate is 2e-2.
  - per (plane d, channel-half mh, h-block hb): ONE bf16 matmul
    [108,96]^T @ [108, 8x64] -> PSUM [96,512] (fp32 accumulate).
  - PSUM evacuated with fused per-channel bias add, alternating DVE/ACT,
    into a [96, 4096] bf16 stage tile; one 768 KB DMA per (d, mh) on the
    ACT HWDGE ring writes y[mh*96:(mh+1)*96, d].
"""

import itertools
from contextlib import ExitStack

import numpy as np

# ---------------------------------------------------------------- constants
CIN = 4
COUT = 8
N_ROT = 24
KS = 3
DHW = 64
PH = 66            # padded plane side
PLANE = PH * PH    # 4356
SLOT = 4224        # ring slot elems per partition (= 64*66)
VALID = 4222       # elems actually written per partition (max read idx 4221)
N_CORES = 8
DCHUNK = 16        # output planes per core
SLAB_D = 18        # input padded planes per core (16 + 2 halo)
M = 192            # total output channels (24 rot * 8 cout)
MHALF = 96
NTAP = 36          # (kh,kw,ci) rows per kd  == ring block height
HB = 8             # h-blocks per plane
NCOL = 512         # matmul free dim (8 h-rows * 64)

_CACHE = {}


def _rot_index_maps(k):
    """Source voxel indices (d,h,w) for the 24 proper octahedral rotations."""
    m = (k - 1) // 2
    mats = []
    for perm in itertools.permutations(range(3)):
        for signs in itertools.product([1, -1], repeat=3):
            R = np.zeros((3, 3))
            for i in range(3):
                R[i, perm[i]] = signs[i]
            if np.linalg.det(R) > 0.5:
                mats.append(R)
    c = np.arange(k) - m
    cz, cy, cx = np.meshgrid(c, c, c, indexing="ij")
    v = np.stack([cx, cy, cz], 0).astype(np.float64)
    idx = []
    for R in mats:
        g = np.einsum("ij,jdhw->idhw", R, v)
        idx.append(
            np.stack([g[2] + m, g[1] + m, g[0] + m], 0).round().astype(np.int64)
        )
    return np.stack(idx, 0)  # (24, 3, k, k, k)


def _build_program(repeat=1, mode="full"):
    import concourse.bacc as bacc
    import concourse.bass as bass
    import concourse.mybir as mybir
    import concourse.tile as tile

    f32 = mybir.dt.float32
    nc = bacc.Bacc(
        "TRN2",
        target_bir_lowering=False,
        debug=False,
        enable_asserts=False,
        num_devices=N_CORES,
    )

    bf16 = mybir.dt.bfloat16
    xs = nc.dram_tensor(
        "xs", (NTAP, SLAB_D, VALID), bf16, kind="ExternalInput"
    )
    w3 = nc.dram_tensor("w3", (KS * NTAP, 3 * M), bf16, kind="ExternalInput")
    bias2 = nc.dram_tensor("bias2", (MHALF, 2), f32, kind="ExternalInput")
    y = nc.dram_tensor("y", (M, DCHUNK, DHW, DHW), bf16, kind="ExternalOutput")
    xs_ap = xs.ap()
    y_ap = y.ap()

    with tile.TileContext(nc) as tc:
        with ExitStack() as ctx:
            wpool = ctx.enter_context(tc.tile_pool(name="wpool", bufs=1))
            psum = ctx.enter_context(
                tc.tile_pool(name="psum", bufs=8, space="PSUM")
            )
            stage = ctx.enter_context(tc.tile_pool(name="stage", bufs=6))

            # ---- weights + bias (tiny, once)
            w3_t = wpool.tile([KS * NTAP, 3 * M], bf16)
            nc.sync.dma_start(w3_t[:], w3.ap())
            bias_t = wpool.tile([MHALF, 2], f32)
            nc.sync.dma_start(bias_t[:], bias2.ap())
            # two persistent mod-3 rings (3 partition blocks of NTAP rows
            # each): A covers output planes 0-7 (inputs 0-9), B covers
            # 8-15 (inputs 8-17). Interleaving A/B planes hides each
            # ring's reload latency behind the other ring's matmuls.
            rbufA = wpool.tile([KS * NTAP, SLOT], bf16, name="rbufA")
            rbufB = wpool.tile([KS * NTAP, SLOT], bf16, name="rbufB")

            loop_ctx = ExitStack()
            if repeat > 1:
                loop_ctx.enter_context(
                    tc.For_i(
                        0,
                        repeat,
                        1,
                        hint_engines=(
                            mybir.EngineType.PE,
                            mybir.EngineType.DVE,
                            mybir.EngineType.Activation,
                            mybir.EngineType.SP,
                            mybir.EngineType.Pool,
                        ),
                    )
                )

            def load_block(buf, q):
                # SWDGE on the otherwise-idle Pool sequencer: the SP/ACT
                # HWDGE sequencers serialize ~5.5 us per DMA (issue ->
                # completion), which was the dominant serial cost.
                b = q % 3
                nc.gpsimd.dma_start(
                    buf[NTAP * b : NTAP * (b + 1), 0:VALID], xs_ap[:, q, :]
                )

            noload = mode.endswith("noload")

            for q in range(3):
                load_block(rbufA, q)
            for q in range(8, 11):
                load_block(rbufB, q)

            rhsA = rbufA[:].rearrange("p (h w) -> p h w", w=PH)
            rhsB = rbufB[:].rearrange("p (h w) -> p h w", w=PH)

            def emit_plane(rhs1, d):
                r = d % 3
                for mh in range(2):
                    st = stage.tile([MHALF, HB * NCOL], bf16, tag="stage")
                    for hb in range(HB):
                        ps = psum.tile([MHALF, NCOL], f32, tag="ps")
                        lhsT = w3_t[
                            :, r * M + mh * MHALF : r * M + (mh + 1) * MHALF
                        ]
                        rhs = rhs1[:, hb * HB : hb * HB + HB, 0:DHW]
                        nc.tensor.matmul(
                            ps[:],
                            lhsT,
                            rhs,
                            start=True,
                            stop=True,
                        )
                        if mode.startswith("mm"):
                            continue
                        if mode != "noevac":
                            dst = st[:, hb * NCOL : (hb + 1) * NCOL]
                            if (mh * HB + hb) % 2 == 0:
                                nc.vector.tensor_scalar_add(
                                    dst, ps[:], bias_t[:, mh : mh + 1]
                                )
                            else:
                                nc.scalar.activation(
                                    dst,
                                    ps[:],
                                    mybir.ActivationFunctionType.Identity,
                                    bias=bias_t[:, mh : mh + 1],
                                )
                    if mode == "noevac":
                        # keep the stage tile written so the out-DMA has data
                        nc.vector.tensor_scalar_add(
                            st[:, 0:NCOL], ps[:], bias_t[:, mh : mh + 1]
                        )
                    if not mode.startswith("mm") and mode != "noout":
                        nc.scalar.dma_start(
                            y_ap[mh * MHALF : (mh + 1) * MHALF, d], st[:]
                        )

            for j in range(8):
                emit_plane(rhsA, j)
                if j + 3 <= 9 and not noload:
                    load_block(rbufA, j + 3)
                emit_plane(rhsB, 8 + j)
                if j + 3 <= 9 and not noload:
                    load_block(rbufB, 11 + j)

            loop_ctx.close()

    nc.compile()
    return nc


def _make_runner(nc):
    """Build a reusable jitted SPMD executor (no donation so device buffers
    can be reused across timing calls). Modeled on bass2jax.run_bass_via_pjrt."""
    import jax
    import numpy as _np
    from jax.sharding import Mesh, PartitionSpec
    from jax.experimental.shard_map import shard_map

    import concourse.mybir as mybir
    from concourse import bass2jax

    bass2jax.install_neuronx_cc_hook()

    partition_name = (
        nc.partition_id_tensor.name if nc.partition_id_tensor else None
    )
    in_names, out_names, out_avals, zero_outs = [], [], [], []
    for alloc in nc.m.functions[0].allocations:
        if not isinstance(alloc, mybir.MemoryLocationSet):
            continue
        name = alloc.memorylocations[0].name
        if alloc.kind == "ExternalInput":
            if name != partition_name:
                in_names.append(name)
        elif alloc.kind == "ExternalOutput":
            shape = tuple(alloc.tensor_shape)
            dtype = mybir.dt.np(alloc.dtype)
            out_names.append(name)
            out_avals.append(jax.core.ShapedArray(shape, dtype))
            zero_outs.append(_np.zeros(shape, dtype))
    n_params = len(in_names)
    all_names = in_names + out_names
    if partition_name is not None:
        all_names = all_names + [partition_name]

    def _body(*args):
        operands = list(args)
        if partition_name is not None:
            operands.append(bass2jax.partition_id_tensor())
        outs = bass2jax._bass_exec_p.bind(
            *operands,
            out_avals=tuple(out_avals),
            in_names=tuple(all_names),
            out_names=tuple(out_names),
            lowering_input_output_aliases=(),
            sim_require_finite=True,
            sim_require_nnan=True,
            nc=nc,
        )
        return tuple(outs)

    devices = jax.devices()[:N_CORES]
    mesh = Mesh(np.asarray(devices), ("core",))
    n_args = n_params + len(out_names)
    sharded = jax.jit(
        shard_map(
            _body,
            mesh=mesh,
            in_specs=(PartitionSpec("core"),) * n_args,
            out_specs=(PartitionSpec("core"),) * len(out_names),
            check_rep=False,
        ),
        keep_unused=True,
    )

    from jax.sharding import NamedSharding

    shard = NamedSharding(mesh, PartitionSpec("core"))

    def place_inputs(in_maps):
        """Device-put per-core inputs (sharded along axis 0) + cached zero
        output buffers; returns the full arg list, all device-resident."""
        concat = [
            np.concatenate([np.asarray(m[name]) for m in in_maps], axis=0)
            for name in in_names
        ]
        placed = [jax.device_put(a, shard) for a in concat]
        if "zeros" not in _CACHE:
            _CACHE["zeros"] = [
                jax.device_put(
                    np.zeros((N_CORES * z.shape[0], *z.shape[1:]), z.dtype), shard
                )
                for z in zero_outs
            ]
        return placed + _CACHE["zeros"]

    def run(args):
        return sharded(*args)

    return {
        "place_inputs": place_inputs,
        "run": run,
        "out_names": out_names,
        "out_avals": out_avals,
    }


def _get_runner():
    if "runner" not in _CACHE:
        nc = _build_program()
        _CACHE["runner"] = _make_runner(nc)
    return _CACHE["runner"]


def _get_timing_runner(repeat, mode="full"):
    key = f"runner_r{repeat}_{mode}"
    if key not in _CACHE:
        nc = _build_program(repeat=repeat, mode=mode)
        _CACHE[key] = _make_runner(nc)
    return _CACHE[key]


def _host_prep(x, weight, bias):
    import ml_dtypes

    bf16 = ml_dtypes.bfloat16
    idx = _rot_index_maps(KS)
    wr = weight[:, :, idx[:, 0], idx[:, 1], idx[:, 2]]  # (8,4,24,3,3,3)
    wr = np.transpose(wr, (2, 0, 1, 3, 4, 5)).reshape(M, CIN, KS, KS, KS)
    # lhsT variant r = d%3: ring block b holds kd = (b - r) % 3; rows within
    # a block ordered (kh, kw, ci) to match the xs9 partition order.
    lhsT3 = np.empty((3, KS * NTAP, M), dtype=np.float32)
    for r in range(3):
        for b in range(3):
            kd = (b - r) % 3
            lhsT3[r, NTAP * b : NTAP * (b + 1)] = (
                wr[:, :, kd].transpose(2, 3, 1, 0).reshape(NTAP, M)
            )
    w3 = np.ascontiguousarray(
        lhsT3.transpose(1, 0, 2).reshape(KS * NTAP, 3 * M)
    ).astype(bf16)
    bias192 = np.broadcast_to(bias[None, :], (N_ROT, COUT)).reshape(M)
    bias2 = np.ascontiguousarray(bias192.reshape(2, MHALF).T, dtype=np.float32)

    x_pad = np.zeros((2, CIN, PH, PH, PH), dtype=bf16)
    x_pad[:, :, 1:65, 1:65, 1:65] = x.astype(bf16)

    in_maps = []
    for core in range(N_CORES):
        n, dc = divmod(core, N_CORES // 2)
        flat = x_pad[n, :, DCHUNK * dc : DCHUNK * dc + SLAB_D].reshape(
            CIN, SLAB_D, PLANE
        )
        col9 = np.empty((KS * KS, CIN, SLAB_D, VALID), dtype=bf16)
        for kh in range(KS):
            for kw in range(KS):
                off = kh * PH + kw
                col9[kh * KS + kw] = flat[:, :, off : off + VALID]
        xs9 = np.ascontiguousarray(col9.reshape(NTAP, SLAB_D, VALID))
        in_maps.append({"xs": xs9, "w3": w3, "bias2": bias2})
    return in_maps


def kernel(x, weight, bias):
    x = np.asarray(x, dtype=np.float32)
    weight = np.asarray(weight, dtype=np.float32)
    bias = np.asarray(bias, dtype=np.float32)

    runner = _get_runner()
    in_maps = _host_prep(x, weight, bias)
    args = runner["place_inputs"](in_maps)
    out = runner["run"](args)
    y8 = np.asarray(out[0]).reshape(N_CORES, M, DCHUNK, DHW, DHW)

    yfull = np.empty((2, M, DHW, DHW, DHW), dtype=np.float32)
    for core in range(N_CORES):
        n, dc = divmod(core, N_CORES // 2)
        yfull[n, :, DCHUNK * dc : DCHUNK * (dc + 1)] = y8[core]
    return yfull.reshape(2, N_ROT, COUT, DHW, DHW, DHW)


# revision 12
# speedup vs baseline: 1.6565x; 1.6558x over previous
"""Trainium2 Bass kernel for 24-rotation (octahedral) 3D conv (ConvZ3P24).

Problem: x (2,4,64,64,64) f32, weight (8,4,3,3,3), bias (8,)
  -> y (2,24,8,64,64,64):  conv3d(x, rotated_filter_bank) + bias,
  stride 1, pad 1, 24 proper octahedral rotations x 8 cout = 192 channels.

Sharding: 8 cores = batch(2) x depth-chunks(4 x 16 planes). Each core
computes all 192 channels for its 16 output planes.

Device kernel (per core), conv as implicit GEMM with K=108 packed taps,
HBM traffic minimized two ways vs the naive im2col formulation:
  1. mod-3 depth ring: the rhs [108, SLOT] SBUF buffer is 3 partition
     blocks of 36 rows (kh,kw,ci); block b holds the (kh,kw)-shifted
     copies of padded input plane q with q === b (mod 3). Each loaded
     plane-block serves 3 consecutive output planes, so the host-side
     im2col only replicates 9x (kh,kw), not 27x: xs9 [36, 18, 4222]
     (5.5 MB/core vs 14.6 MB). The lhsT has 3 column variants r = d%3
     with the kd blocks permuted to match the ring phase.
  2. bf16 output: y is stored bf16 (25.2 MB/core vs 50.3 MB f32) and
     upcast on the host. Adds <=0.4% scale-relative error; gate is 2e-2.
  - per (plane d, channel-half mh, h-block hb): ONE bf16 matmul
    [108,96]^T @ [108, 8x64] -> PSUM [96,512] (fp32 accumulate).
  - PSUM evacuated with fused per-channel bias add, alternating DVE/ACT,
    into a [96, 4096] bf16 stage tile; one 768 KB DMA per (d, mh) on the
    ACT HWDGE ring writes y[mh*96:(mh+1)*96, d].
"""

import itertools
from contextlib import ExitStack

import numpy as np

# ---------------------------------------------------------------- constants
CIN = 4
COUT = 8
N_ROT = 24
KS = 3
DHW = 64
PH = 66            # padded plane side
PLANE = PH * PH    # 4356
SLOT = 4224        # ring slot elems per partition (= 64*66)
VALID = 4222       # elems actually written per partition (max read idx 4221)
N_CORES = 8
DCHUNK = 16        # output planes per core
SLAB_D = 18        # input padded planes per core (16 + 2 halo)
M = 192            # total output channels (24 rot * 8 cout)
MHALF = 96
NTAP = 36          # (kh,kw,ci) rows per kd  == ring block height
HB = 8             # h-blocks per plane
NCOL = 512         # matmul free dim (8 h-rows * 64)

_CACHE = {}


def _rot_index_maps(k):
    """Source voxel indices (d,h,w) for the 24 proper octahedral rotations."""
    m = (k - 1) // 2
    mats = []
    for perm in itertools.permutations(range(3)):
        for signs in itertools.product([1, -1], repeat=3):
            R = np.zeros((3, 3))
            for i in range(3):
                R[i, perm[i]] = signs[i]
            if np.linalg.det(R) > 0.5:
                mats.append(R)
    c = np.arange(k) - m
    cz, cy, cx = np.meshgrid(c, c, c, indexing="ij")
    v = np.stack([cx, cy, cz], 0).astype(np.float64)
    idx = []
    for R in mats:
        g = np.einsum("ij,jdhw->idhw", R, v)
        idx.append(
            np.stack([g[2] + m, g[1] + m, g[0] + m], 0).round().astype(np.int64)
        )
    return np.stack(idx, 0)  # (24, 3, k, k, k)


def _build_program(repeat=1, mode="full"):
    import concourse.bacc as bacc
    import concourse.bass as bass
    import concourse.mybir as mybir
    import concourse.tile as tile

    f32 = mybir.dt.float32
    nc = bacc.Bacc(
        "TRN2",
        target_bir_lowering=False,
        debug=False,
        enable_asserts=False,
        num_devices=N_CORES,
    )

    bf16 = mybir.dt.bfloat16
    xs = nc.dram_tensor(
        "xs", (NTAP, SLAB_D, VALID), bf16, kind="ExternalInput"
    )
    w3 = nc.dram_tensor("w3", (KS * NTAP, 3 * M), bf16, kind="ExternalInput")
    bias2 = nc.dram_tensor("bias2", (MHALF, 2), f32, kind="ExternalInput")
    y = nc.dram_tensor("y", (M, DCHUNK, DHW, DHW), bf16, kind="ExternalOutput")
    xs_ap = xs.ap()
    y_ap = y.ap()

    with tile.TileContext(nc) as tc:
        with ExitStack() as ctx:
            wpool = ctx.enter_context(tc.tile_pool(name="wpool", bufs=1))
            psum = ctx.enter_context(
                tc.tile_pool(name="psum", bufs=4, space="PSUM")
            )
            stage = ctx.enter_context(tc.tile_pool(name="stage", bufs=6))

            # ---- weights + bias (tiny, once)
            w3_t = wpool.tile([KS * NTAP, 3 * M], bf16)
            nc.sync.dma_start(w3_t[:], w3.ap())
            bias_t = wpool.tile([MHALF, 2], f32)
            nc.sync.dma_start(bias_t[:], bias2.ap())
            # two persistent mod-3 rings (3 partition blocks of NTAP rows
            # each): A covers output planes 0-7 (inputs 0-9), B covers
            # 8-15 (inputs 8-17). Interleaving A/B planes hides each
            # ring's reload latency behind the other ring's matmuls.
            rbufA = wpool.tile([KS * NTAP, SLOT], bf16, name="rbufA")
            rbufB = wpool.tile([KS * NTAP, SLOT], bf16, name="rbufB")

            loop_ctx = ExitStack()
            if repeat > 1:
                loop_ctx.enter_context(
                    tc.For_i(
                        0,
                        repeat,
                        1,
                        hint_engines=(
                            mybir.EngineType.PE,
                            mybir.EngineType.DVE,
                            mybir.EngineType.Activation,
                            mybir.EngineType.SP,
                            mybir.EngineType.Pool,
                        ),
                    )
                )

            def load_block(buf, q):
                # SWDGE on the otherwise-idle Pool sequencer: the SP/ACT
                # HWDGE sequencers serialize ~5.5 us per DMA (issue ->
                # completion), which was the dominant serial cost.
                b = q % 3
                nc.gpsimd.dma_start(
                    buf[NTAP * b : NTAP * (b + 1), 0:VALID], xs_ap[:, q, :]
                )

            noload = mode.endswith("noload")

            for q in range(3):
                load_block(rbufA, q)
            for q in range(8, 11):
                load_block(rbufB, q)

            rhsA = rbufA[:].rearrange("p (h w) -> p h w", w=PH)
            rhsB = rbufB[:].rearrange("p (h w) -> p h w", w=PH)

            PLCOL = HB * NCOL  # 4096 columns per plane per channel-half

            def emit_plane(rhs1, d, stages, half):
                """One output plane: 16 matmuls into 2-bank PSUM pairs, one
                evac per pair into `half` of the 2-plane stage tiles."""
                r = d % 3
                ps = None
                for mh in range(2):
                    st = stages[mh]
                    for hp in range(4):  # pairs of h-blocks
                        ps = psum.tile([MHALF, 2 * NCOL], f32, tag="ps")
                        lhsT = w3_t[
                            :, r * M + mh * MHALF : r * M + (mh + 1) * MHALF
                        ]
                        for i in range(2):
                            hb = hp * 2 + i
                            rhs = rhs1[:, hb * HB : hb * HB + HB, 0:DHW]
                            nc.tensor.matmul(
                                ps[:, i * NCOL : (i + 1) * NCOL],
                                lhsT,
                                rhs,
                                start=True,
                                stop=True,
                            )
                        if mode.startswith("mm") or mode == "noevac":
                            continue
                        dst = st[
                            :,
                            half * PLCOL + hp * 2 * NCOL :
                            half * PLCOL + (hp + 1) * 2 * NCOL,
                        ]
                        # ACT is faster per column (0.83 vs 1.04 ns): 5/3 split
                        if (mh * 4 + hp) in (1, 4, 6):
                            nc.vector.tensor_scalar_add(
                                dst, ps[:], bias_t[:, mh : mh + 1]
                            )
                        else:
                            nc.scalar.activation(
                                dst,
                                ps[:],
                                mybir.ActivationFunctionType.Identity,
                                bias=bias_t[:, mh : mh + 1],
                            )
                if mode == "noevac" and ps is not None:
                    for mh in range(2):
                        nc.vector.tensor_scalar_add(
                            stages[mh][:, 0 : 2 * NCOL],
                            ps[:],
                            bias_t[:, mh : mh + 1],
                        )

            def store_pair(stages, d0):
                # one 1.5 MB DMA per channel-half covering planes d0, d0+1
                for mh in range(2):
                    nc.sync.dma_start(
                        y_ap[mh * MHALF : (mh + 1) * MHALF, d0 : d0 + 2],
                        stages[mh][:],
                    )

            stagesA = stagesB = None
            for j in range(8):
                if j % 2 == 0:
                    stagesA = [
                        stage.tile([MHALF, 2 * PLCOL], bf16, tag="stage",
                                   name=f"stA{j}_{mh}")
                        for mh in range(2)
                    ]
                    stagesB = [
                        stage.tile([MHALF, 2 * PLCOL], bf16, tag="stage",
                                   name=f"stB{j}_{mh}")
                        for mh in range(2)
                    ]
                emit_plane(rhsA, j, stagesA, j % 2)
                if j + 3 <= 9 and not noload:
                    load_block(rbufA, j + 3)
                if j % 2 == 1 and not mode.startswith("mm") and mode != "noout":
                    store_pair(stagesA, j - 1)
                emit_plane(rhsB, 8 + j, stagesB, j % 2)
                if j + 3 <= 9 and not noload:
                    load_block(rbufB, 11 + j)
                if j % 2 == 1 and not mode.startswith("mm") and mode != "noout":
                    store_pair(stagesB, 8 + j - 1)

            loop_ctx.close()

    nc.compile()
    return nc


def _make_runner(nc):
    """Build a reusable jitted SPMD executor (no donation so device buffers
    can be reused across timing calls). Modeled on bass2jax.run_bass_via_pjrt."""
    import jax
    import numpy as _np
    from jax.sharding import Mesh, PartitionSpec
    from jax.experimental.shard_map import shard_map

    import concourse.mybir as mybir
    from concourse import bass2jax

    bass2jax.install_neuronx_cc_hook()

    partition_name = (
        nc.partition_id_tensor.name if nc.partition_id_tensor else None
    )
    in_names, out_names, out_avals, zero_outs = [], [], [], []
    for alloc in nc.m.functions[0].allocations:
        if not isinstance(alloc, mybir.MemoryLocationSet):
            continue
        name = alloc.memorylocations[0].name
        if alloc.kind == "ExternalInput":
            if name != partition_name:
                in_names.append(name)
        elif alloc.kind == "ExternalOutput":
            shape = tuple(alloc.tensor_shape)
            dtype = mybir.dt.np(alloc.dtype)
            out_names.append(name)
            out_avals.append(jax.core.ShapedArray(shape, dtype))
            zero_outs.append(_np.zeros(shape, dtype))
    n_params = len(in_names)
    all_names = in_names + out_names
    if partition_name is not None:
        all_names = all_names + [partition_name]

    def _body(*args):
        operands = list(args)
        if partition_name is not None:
            operands.append(bass2jax.partition_id_tensor())
        outs = bass2jax._bass_exec_p.bind(
            *operands,
            out_avals=tuple(out_avals),
            in_names=tuple(all_names),
            out_names=tuple(out_names),
            lowering_input_output_aliases=(),
            sim_require_finite=True,
            sim_require_nnan=True,
            nc=nc,
        )
        return tuple(outs)

    devices = jax.devices()[:N_CORES]
    mesh = Mesh(np.asarray(devices), ("core",))
    n_args = n_params + len(out_names)
    sharded = jax.jit(
        shard_map(
            _body,
            mesh=mesh,
            in_specs=(PartitionSpec("core"),) * n_args,
            out_specs=(PartitionSpec("core"),) * len(out_names),
            check_rep=False,
        ),
        keep_unused=True,
    )

    from jax.sharding import NamedSharding

    shard = NamedSharding(mesh, PartitionSpec("core"))

    def place_inputs(in_maps):
        """Device-put per-core inputs (sharded along axis 0) + cached zero
        output buffers; returns the full arg list, all device-resident."""
        concat = [
            np.concatenate([np.asarray(m[name]) for m in in_maps], axis=0)
            for name in in_names
        ]
        placed = [jax.device_put(a, shard) for a in concat]
        if "zeros" not in _CACHE:
            _CACHE["zeros"] = [
                jax.device_put(
                    np.zeros((N_CORES * z.shape[0], *z.shape[1:]), z.dtype), shard
                )
                for z in zero_outs
            ]
        return placed + _CACHE["zeros"]

    def run(args):
        return sharded(*args)

    return {
        "place_inputs": place_inputs,
        "run": run,
        "out_names": out_names,
        "out_avals": out_avals,
    }


def _get_runner():
    if "runner" not in _CACHE:
        nc = _build_program()
        _CACHE["runner"] = _make_runner(nc)
    return _CACHE["runner"]


def _get_timing_runner(repeat, mode="full"):
    key = f"runner_r{repeat}_{mode}"
    if key not in _CACHE:
        nc = _build_program(repeat=repeat, mode=mode)
        _CACHE[key] = _make_runner(nc)
    return _CACHE[key]


def _host_prep(x, weight, bias):
    import ml_dtypes

    bf16 = ml_dtypes.bfloat16
    idx = _rot_index_maps(KS)
    wr = weight[:, :, idx[:, 0], idx[:, 1], idx[:, 2]]  # (8,4,24,3,3,3)
    wr = np.transpose(wr, (2, 0, 1, 3, 4, 5)).reshape(M, CIN, KS, KS, KS)
    # lhsT variant r = d%3: ring block b holds kd = (b - r) % 3; rows within
    # a block ordered (kh, kw, ci) to match the xs9 partition order.
    lhsT3 = np.empty((3, KS * NTAP, M), dtype=np.float32)
    for r in range(3):
        for b in range(3):
            kd = (b - r) % 3
            lhsT3[r, NTAP * b : NTAP * (b + 1)] = (
                wr[:, :, kd].transpose(2, 3, 1, 0).reshape(NTAP, M)
            )
    w3 = np.ascontiguousarray(
        lhsT3.transpose(1, 0, 2).reshape(KS * NTAP, 3 * M)
    ).astype(bf16)
    bias192 = np.broadcast_to(bias[None, :], (N_ROT, COUT)).reshape(M)
    bias2 = np.ascontiguousarray(bias192.reshape(2, MHALF).T, dtype=np.float32)

    x_pad = np.zeros((2, CIN, PH, PH, PH), dtype=bf16)
    x_pad[:, :, 1:65, 1:65, 1:65] = x.astype(bf16)

    in_maps = []
    for core in range(N_CORES):
        n, dc = divmod(core, N_CORES // 2)
        flat = x_pad[n, :, DCHUNK * dc : DCHUNK * dc + SLAB_D].reshape(
            CIN, SLAB_D, PLANE
        )
        col9 = np.empty((KS * KS, CIN, SLAB_D, VALID), dtype=bf16)
        for kh in range(KS):
            for kw in range(KS):
                off = kh * PH + kw
                col9[kh * KS + kw] = flat[:, :, off : off + VALID]
        xs9 = np.ascontiguousarray(col9.reshape(NTAP, SLAB_D, VALID))
        in_maps.append({"xs": xs9, "w3": w3, "bias2": bias2})
    return in_maps


def kernel(x, weight, bias):
    x = np.asarray(x, dtype=np.float32)
    weight = np.asarray(weight, dtype=np.float32)
    bias = np.asarray(bias, dtype=np.float32)

    runner = _get_runner()
    in_maps = _host_prep(x, weight, bias)
    args = runner["place_inputs"](in_maps)
    out = runner["run"](args)
    y8 = np.asarray(out[0]).reshape(N_CORES, M, DCHUNK, DHW, DHW)

    yfull = np.empty((2, M, DHW, DHW, DHW), dtype=np.float32)
    for core in range(N_CORES):
        n, dc = divmod(core, N_CORES // 2)
        yfull[n, :, DCHUNK * dc : DCHUNK * (dc + 1)] = y8[core]
    return yfull.reshape(2, N_ROT, COUT, DHW, DHW, DHW)
